# revision 16
# baseline (speedup 1.0000x reference)
"""NNUE evaluation kernel for Trainium2 (8 NeuronCores, data-parallel batch).

reference math:
    wh = clip(white @ W_ft.T, 0, 1)        # [B, 256]
    bh = clip(black @ W_ft.T, 0, 1)        # [B, 256]
    x  = concat(wh, bh)                    # [B, 512]
    x  = relu(x @ W1.T + b1); x = relu(x @ W2.T + b2)
    ev = (x @ W3.T + b3) * stm[:, None]    # [B, 1]

Strategy: shard B=4096 across 8 cores (512 rows each), data-parallel, no
collectives. The fp16 version of this kernel was DMA-wall-bound: 105 MB
of feature+weight traffic per core at the ~358 GB/s per-NC HBM rate is
293 us, above the 276 us fp16 PE roofline. This version streams the
features as uint8 fixed-point (round(f*255), 42 MB instead of 84 MB per
core; rel err 1.13e-2 vs the 2e-2 gate, dominated by the uniform
(1/255)/sqrt(12) quantization noise through the 40960-term contraction)
and converts them to fp16 on-chip: white slabs on the DVE (4.3 us per
8-k-tile slab), black slabs on the scalar/ACT engine (3.5 us), both
under the 6.9 us PE slab cadence. fp8 DoubleRow (the only 2x PE mode)
was ruled out numerically: e4m3's 3-bit mantissa gives ~9.5e-2 rel err
here. W_ft stays fp16 (21 MB; its quantization error is negligible).
Total DMA 63 MB = 176 us, so the kernel runs at the fp16 PE roofline:
1280 matmuls x 216 ns = 276 us.

All DMA triggers ride the sync HWDGE ring (the ACT queue executes
conversions in order, so a dma_start emitted there would stall behind
them and collapse the prefetch pipeline). The PSUM evacuation applies
clip(psum, 0, 255) (the 1/255 dequant scale is folded into W1 on the
host), white's evacuation hides under black's final matmuls, the tiny
MLP stays in transposed [features, batch] layout, and b3 is folded into
the last matmul via a ones-row. Dummy warm-up matmuls cover the HWDGE
bring-up so the HAM clock gate is at 2.4 GHz when real matmuls start.

This walrus build rejects instructions with >1 sync wait, so a post-pass
(_split_multi_waits) redistributes Tile-emitted waits onto single-wait
no-ops.
"""

import sys
import types

import numpy as np


def _inject_ntff_hook():
    """Register the axon NTFF profile hook if this image's antenv lacks it."""
    try:
        import antenv.axon_hooks  # noqa: F401
        return
    except ImportError:
        pass
    try:
        import trn_agent_boot.trn_boot as tb
        hook = tb._ntff_profile_via_ctypes("/opt/axon/libaxon_pjrt.so")
    except Exception:
        hook = None
    mod = types.ModuleType("antenv.axon_hooks")
    mod.get_axon_ntff_profile_hook = lambda: hook
    mod.set_axon_ntff_profile_hook = lambda h: None
    sys.modules["antenv.axon_hooks"] = mod


_inject_ntff_hook()

import concourse.bass as bass
import concourse.mybir as mybir
from concourse.tile import TileContext

N_CORES = 8
B = 4096
BS = B // N_CORES          # 512 batch rows per core
IN = 40960                 # feature count (contraction dim)
H = 256                    # hidden per perspective
NKT = 8                    # k-tiles per slab
KC = NKT * 128             # k-slab width: 1024
NKTOT = IN // 128          # 320 k-tiles total

F32 = mybir.dt.float32
F16 = mybir.dt.float16
U8 = mybir.dt.uint8


def _split_multi_waits(nc: bass.Bass) -> None:
    """This walrus build rejects instructions carrying more than one sync
    wait. Split any such instruction: emit single-wait no-ops on the same
    engine immediately before it (same engine stream => same semantics)."""
    for f in nc.m.functions:
        for bb in f.blocks:
            new_insts = []
            changed = False
            for inst in bb.instructions:
                si = inst.sync_info
                waits = list(si.on_wait) if si is not None and si.on_wait else []
                if len(waits) > 1:
                    changed = True
                    for i, w in enumerate(waits[:-1]):
                        nop = mybir.InstNoOp(
                            name=f"{inst.name}-sw{i}", ins=[], outs=[]
                        )
                        nop.engine = inst.engine
                        nop.sync_info = mybir.SyncInfo(on_wait=[w], on_update=[])
                        nc.register_instruction(nop)
                        new_insts.append(nop)
                    inst.sync_info = mybir.SyncInfo(
                        on_wait=[waits[-1]],
                        on_update=list(si.on_update) if si.on_update else [],
                    )
                new_insts.append(inst)
            if changed:
                bb.instructions = new_insts


def build_kernel(mm_f32r: bool = True, tr_f32r: bool = True) -> bass.Bass:
    nc = bass.Bass()

    # Features arrive host-quantized uint8 (round(f*255)) and
    # host-transposed/swizzled: [128, NKTOT*BS] where row p, columns
    # [kt*BS : (kt+1)*BS] hold feat_q.T[kt*128 + p, :]. Each NKT=8 k-slab
    # DMA reads NKT*BS = 4 KB contiguous per partition.
    wf = nc.dram_tensor("white_u8", [128, NKTOT * BS], U8, kind="ExternalInput")
    bf = nc.dram_tensor("black_u8", [128, NKTOT * BS], U8, kind="ExternalInput")
    # W_ft.T swizzled the same way, fp16: [128, NKTOT*H], 4 KB/partition
    # per 8-tile slab.
    w_ftTs = nc.dram_tensor("W_ftTs", [128, NKTOT * H], F16, kind="ExternalInput")
    # W1 carries the 1/255 feature-dequant scale (folded on the host).
    w1Ts = nc.dram_tensor("W1Ts", [128, 128], F16, kind="ExternalInput")
    b1 = nc.dram_tensor("b1", [32, 1], F32, kind="ExternalInput")
    w2T = nc.dram_tensor("W2T", [32, 32], F16, kind="ExternalInput")
    b2 = nc.dram_tensor("b2", [32, 1], F32, kind="ExternalInput")
    # W3T has b3 folded in as a 33rd row (paired with a ones-row in h2)
    w3T = nc.dram_tensor("W3T", [33, 1], F16, kind="ExternalInput")
    stm = nc.dram_tensor("side_to_move", [1, BS], F32, kind="ExternalInput")
    out = nc.dram_tensor("evaluation", [1, BS], F32, kind="ExternalOutput")

    feats = [wf, bf]

    with TileContext(nc) as tc:
        with (
            tc.tile_pool(name="ot_psum", bufs=1, space="PSUM") as ot_pool,
            tc.tile_pool(name="mlp", bufs=1) as mlp,
            # The warm pools stay open for the whole kernel: if they were
            # scoped and released, the feature pools would reuse their SBUF
            # and every feature DMA would inherit a WAR dependency on ALL
            # dummy matmuls (stalling the whole head of the DMA stream
            # behind the PE warm-up).
            tc.tile_pool(name="warm", bufs=1) as warm_pool,
            tc.tile_pool(name="warm_psum", bufs=1, space="PSUM") as wp_pool,
        ):
            # out.T accumulators: [h-tile 128, b 512] x (2 sides x 2 h-tiles)
            ot = [
                ot_pool.tile([128, BS], F32, tag=f"ot{i}", name=f"ot{i}")
                for i in range(4)
            ]
            xt = []  # clipped fp16 copies, filled during the last slab

            # ---- PE warm-up: the HAM clock gate defaults to 1.2 GHz and
            # only lifts to 2.4 GHz after ~3.4us of sustained PE activity.
            # Burn that window on dummy matmuls while the first feature
            # slabs are still in flight, so real matmuls start warm.
            # Memsets run on the otherwise-idle GpSimd engine so the DVE's
            # first queued instruction is the slab-0 convert.
            dum_w = warm_pool.tile([128, 128], F16)
            nc.gpsimd.memset(dum_w[:], 0.0)
            dum_f = warm_pool.tile([128, BS], F16)
            nc.gpsimd.memset(dum_f[:], 0.0)
            # Prime the ACT function tables (Copy for the converts, Relu for
            # the MLP) so no activation pays the ~1.3us table load mid-kernel.
            act_prime = warm_pool.tile([1, 1], F16)
            nc.scalar.copy(out=act_prime[:], in_=dum_w[0:1, 0:1])
            nc.scalar.activation(
                out=act_prime[:], in_=dum_w[0:1, 0:1],
                func=mybir.ActivationFunctionType.Relu)
            dum_o = wp_pool.tile([128, BS], F32)
            for _ in range(11):
                nc.tensor.matmul(
                    dum_o, dum_w[:], dum_f[:], start=True, stop=True
                )

            # ---- main loop: feature-transformer GEMMs ----
            # slab widths in k-tiles: small warmup slabs so the PE starts
            # early (the head is bound by HWDGE trigger serialization at
            # ~0.6us per dma_start plus ~2us completion latency, vs
            # 0.86us/k-tile PE consumption), then uniform NKT-wide slabs.
            widths = [4, 4] + [NKT] * ((NKTOT - 8) // NKT)
            assert sum(widths) == NKTOT
            kt0s = [sum(widths[:i]) for i in range(len(widths))]
            # W rides in chunks matching the slab widths, emitted after the
            # f-DMAs of the slab one before the chunk's first consumer
            # (features first: a W chunk queued ahead of a feature slab on
            # the FIFO ring would delay it).
            cwidths = list(widths)
            ckt0s = list(kt0s)
            chunk_of_kt = {}
            for c, (ck, cw) in enumerate(zip(ckt0s, cwidths)):
                for k in range(ck, ck + cw):
                    chunk_of_kt[k] = (c, k - ck)
            first_consumer = [
                next(s for s, k in enumerate(kt0s) if k >= ck)
                for ck in ckt0s
            ]
            fetch_after = {}
            for c in range(len(cwidths)):
                fetch_after.setdefault(max(first_consumer[c] - 1, 0), []).append(c)
            with (
                tc.tile_pool(name="fu8", bufs=10) as fu8_pool,
                tc.tile_pool(name="fcv", bufs=4) as fcv_pool,
                tc.tile_pool(name="wt", bufs=4) as wt_pool,
            ):
                wt_chunks = {}
                for s, w in enumerate(widths):
                    kt0 = kt0s[s]
                    last_slab = s == len(widths) - 1
                    fsl = [None, None]
                    # black's DMA first: its convert runs on the slower ACT
                    # engine, so it needs its data earlier than white.
                    for side in (1, 0):
                        f_u8 = fu8_pool.tile(
                            [128, NKT, BS], U8, tag=f"fu8{side}",
                            name=f"fu8{side}",
                        )
                        nc.sync.dma_start(
                            out=f_u8[:, :w, :],
                            in_=feats[side][:, kt0 * BS:(kt0 + w) * BS],
                        )
                        f_t = fcv_pool.tile(
                            [128, NKT, BS], F16, tag=f"fcv{side}",
                            name=f"fcv{side}",
                        )
                        fsl[side] = (f_u8, f_t)
                    for c in fetch_after.get(s, ()):
                        cw = cwidths[c]
                        ck = ckt0s[c]
                        wt_c = wt_pool.tile([128, NKT, H], F16, tag="wt",
                                            name="wt")
                        # the first two W chunks ride the otherwise-idle
                        # gpsimd SWDGE ring, in parallel with the feature
                        # HWDGE stream, so the first matmul isn't gated on
                        # W queued behind features.
                        eng = nc.gpsimd if c < 2 else nc.sync
                        eng.dma_start(
                            out=wt_c[:, :cw, :],
                            in_=w_ftTs[:, ck * H:(ck + cw) * H])
                        wt_chunks[c] = wt_c
                    for side in range(2):
                        f_u8, f_t = fsl[side]
                        if side == 0:
                            nc.vector.tensor_scalar_add(
                                out=f_t[:, :w, :], in0=f_u8[:, :w, :],
                                scalar1=0.0)
                        else:
                            nc.scalar.copy(out=f_t[:, :w, :],
                                           in_=f_u8[:, :w, :])

                    def wslice(kt, h):
                        c, off = chunk_of_kt[kt0 + kt]
                        return wt_chunks[c][:, off, h * 128:(h + 1) * 128]

                    if not last_slab:
                        for kt in range(w):
                            first = kt0 == 0 and kt == 0
                            for h in range(2):
                                for side in range(2):
                                    nc.tensor.matmul(
                                        ot[side * 2 + h],
                                        wslice(kt, h),
                                        fsl[side][1][:, kt, :],
                                        start=first,
                                        stop=False,
                                    )
                    else:
                        # final slab: finish white first, evacuate its
                        # PSUM banks while black's last matmuls run.
                        for side in range(2):
                            for kt in range(w):
                                for h in range(2):
                                    nc.tensor.matmul(
                                        ot[side * 2 + h],
                                        wslice(kt, h),
                                        fsl[side][1][:, kt, :],
                                        start=False,
                                        stop=kt == w - 1,
                                    )
                            for i in range(2 * side, 2 * side + 2):
                                t = mlp.tile([128, BS], F16, tag=f"xt{i}",
                                             name="xt")
                                xt.append(t)
                            # white: full evacs (hide under black's final
                            # matmuls). black: quarters, interleaved by
                            # column so each MLP quarter starts as soon as
                            # its columns land.
                            nsp = 1 if side == 0 else 4
                            for sp in range(nsp):
                                sl = slice(sp * (BS // nsp),
                                           (sp + 1) * (BS // nsp))
                                for i in range(2 * side, 2 * side + 2):
                                    nc.vector.tensor_scalar(
                                        out=xt[i][:, sl], in0=ot[i][:, sl],
                                        scalar1=0.0, scalar2=255.0,
                                        op0=mybir.AluOpType.max,
                                        op1=mybir.AluOpType.min,
                                    )

            # ---- MLP weight prep (emitted late so these DMAs queue on the
            # sync ring behind the feature stream, not ahead of it) ----
            w1t = mlp.tile([128, 4, 32], F16)
            nc.sync.dma_start(out=w1t[:], in_=w1Ts[:, :])
            w2t = mlp.tile([32, 32], F16)
            nc.sync.dma_start(out=w2t[:], in_=w2T[:, :])
            w3t = mlp.tile([33, 1], F16)
            nc.sync.dma_start(out=w3t[:], in_=w3T[:, :])
            b1_sb = mlp.tile([32, 1], F32)
            nc.sync.dma_start(out=b1_sb[:], in_=b1[:, :])
            b2_sb = mlp.tile([32, 1], F32)
            nc.sync.dma_start(out=b2_sb[:], in_=b2[:, :])
            stm_sb = mlp.tile([1, BS], F32)
            nc.sync.dma_start(out=stm_sb[:], in_=stm[:, :])
            # h2 carries a ones-row (partition 32) so the final matmul
            # against [W3.T; b3] folds the bias in.
            h2 = mlp.tile([33, BS], F16)
            nc.vector.memset(h2[32:33, :], 1.0)

            # ---- MLP (transposed layout throughout; xt built above).
            # The chain runs in four 128-column quarters with independent
            # PSUM groups; the h1 bias+relu runs on ACT and the h2 one on
            # DVE so evacuations of one quarter pipeline with PE matmuls
            # of the next. The output DMA goes out in halves so the first
            # half's completion latency hides under the second half's
            # compute. ----
            with tc.tile_pool(name="mlp2_psum", bufs=1, space="PSUM") as mpp2:
                h1p = mpp2.tile([32, BS], F32, tag="h1")
                h1 = mlp.tile([32, BS], F16)
                h2p = mpp2.tile([32, BS], F32, tag="h2")
                evp = mpp2.tile([1, BS], F32, tag="ev")
                evs = mlp.tile([1, BS], F32)
                QB = BS // 4
                for q in range(4):
                    sl = slice(q * QB, (q + 1) * QB)
                    for kt in range(4):
                        nc.tensor.matmul(
                            h1p[:, sl], w1t[:, kt, :], xt[kt][:, sl],
                            start=kt == 0, stop=kt == 3,
                        )
                    nc.scalar.activation(
                        out=h1[:, sl], in_=h1p[:, sl],
                        func=mybir.ActivationFunctionType.Relu,
                        bias=b1_sb[:, :], scale=1.0,
                    )
                    nc.tensor.matmul(
                        h2p[:, sl], w2t[:], h1[:, sl], start=True, stop=True
                    )
                    nc.vector.tensor_scalar(
                        out=h2[0:32, sl], in0=h2p[:, sl], scalar1=b2_sb[:, :],
                        scalar2=0.0,
                        op0=mybir.AluOpType.add, op1=mybir.AluOpType.max,
                    )
                    nc.tensor.matmul(
                        evp[:, sl], w3t[:], h2[:, sl], start=True, stop=True
                    )
                    nc.vector.tensor_mul(
                        out=evs[:, sl], in0=evp[:, sl], in1=stm_sb[:, sl])
                    if q % 2 == 1:
                        hsl = slice((q - 1) * QB, (q + 1) * QB)
                        nc.sync.dma_start(out=out[:, hsl], in_=evs[:, hsl])

    _split_multi_waits(nc)
    return nc


_NC_CACHE: dict = {}


def _get_nc(mm_f32r: bool = True, tr_f32r: bool = True) -> bass.Bass:
    key = (mm_f32r, tr_f32r)
    if key not in _NC_CACHE:
        _NC_CACHE[key] = build_kernel(mm_f32r=mm_f32r, tr_f32r=tr_f32r)
    return _NC_CACHE[key]


def _swizzle_T(arr: np.ndarray, ncols: int) -> np.ndarray:
    """[rows, IN] -> [128, NKTOT*rows] where row p, cols
    [kt*rows:(kt+1)*rows] = arr.T[kt*128 + p, :]."""
    rows = arr.shape[0]
    assert arr.shape == (rows, IN) and ncols == rows
    return np.ascontiguousarray(
        arr.reshape(rows, NKTOT, 128).transpose(2, 1, 0)
    ).reshape(128, NKTOT * rows)


def make_in_maps(inputs: dict) -> list:
    """Shard full inputs into per-core input maps (u8 features, fp16 W)."""
    wf = np.asarray(inputs["white_features"], dtype=np.float32)
    bf = np.asarray(inputs["black_features"], dtype=np.float32)
    wf_q = np.rint(wf * 255.0).astype(np.uint8)
    bf_q = np.rint(bf * 255.0).astype(np.uint8)
    stm = np.ascontiguousarray(inputs["side_to_move"], dtype=np.float32)
    w_ftTs = _swizzle_T(
        np.asarray(inputs["W_ft"], dtype=np.float32).astype(np.float16), H)
    # fold the 1/255 feature-dequant scale into W1
    w1T = (np.asarray(inputs["W1"], dtype=np.float32) / 255.0).astype(
        np.float16).T
    w1Ts = np.ascontiguousarray(
        w1T.reshape(4, 128, 32).transpose(1, 0, 2)).reshape(128, 128)
    w2T = np.ascontiguousarray(
        np.asarray(inputs["W2"], dtype=np.float32).astype(np.float16).T)
    w3T = np.concatenate([
        np.asarray(inputs["W3"], dtype=np.float32).astype(np.float16).T,
        np.asarray(inputs["b3"], dtype=np.float32).astype(np.float16)
        .reshape(1, 1),
    ], axis=0)  # [33, 1]: W3.T with b3 folded in
    maps = []
    for c in range(N_CORES):
        sl = slice(c * BS, (c + 1) * BS)
        maps.append({
            "white_u8": _swizzle_T(wf_q[sl], BS),
            "black_u8": _swizzle_T(bf_q[sl], BS),
            "side_to_move": stm[sl].reshape(1, BS),
            "W_ftTs": w_ftTs,
            "W1Ts": w1Ts,
            "b1": np.ascontiguousarray(inputs["b1"], dtype=np.float32).reshape(32, 1),
            "W2T": w2T,
            "b2": np.ascontiguousarray(inputs["b2"], dtype=np.float32).reshape(32, 1),
            "W3T": w3T,
        })
    return maps


def run(inputs: dict, trace: bool = False, mm_f32r: bool = True,
        tr_f32r: bool = True):
    """Run on all 8 cores; returns (full_output [4096,1] fp32, BassKernelResults)."""
    from concourse.bass_utils import run_bass_kernel_spmd

    nc = _get_nc(mm_f32r=mm_f32r, tr_f32r=tr_f32r)
    res = run_bass_kernel_spmd(
        nc, make_in_maps(inputs), core_ids=list(range(N_CORES)), trace=trace
    )
    full = np.concatenate(
        [res.results[c]["evaluation"].reshape(BS, 1) for c in range(N_CORES)],
        axis=0,
    ).astype(np.float32)
    return full, res


def kernel(**inputs) -> np.ndarray:
    return run(inputs, trace=False)[0]


if __name__ == "__main__":
    rng = np.random.default_rng(0)
    ins = {
        "white_features": rng.random((B, IN), dtype=np.float32),
        "black_features": rng.random((B, IN), dtype=np.float32),
        "side_to_move": np.ones((B,), dtype=np.float32),
        "W_ft": (0.1 * rng.standard_normal((H, IN))).astype(np.float32),
        "W1": (0.06 * rng.standard_normal((32, 2 * H))).astype(np.float32),
        "b1": np.zeros(32, np.float32),
        "W2": (0.17 * rng.standard_normal((32, 32))).astype(np.float32),
        "b2": np.zeros(32, np.float32),
        "W3": (0.24 * rng.standard_normal((1, 32))).astype(np.float32),
        "b3": np.zeros(1, np.float32),
    }
    out = kernel(**ins)
    # host reference
    whr = np.clip(ins["white_features"] @ ins["W_ft"].T, 0, 1)
    bhr = np.clip(ins["black_features"] @ ins["W_ft"].T, 0, 1)
    x = np.concatenate([whr, bhr], axis=1)
    x = np.maximum(x @ ins["W1"].T + ins["b1"], 0)
    x = np.maximum(x @ ins["W2"].T + ins["b2"], 0)
    ref = (x @ ins["W3"].T + ins["b3"]) * ins["side_to_move"][:, None]
    rel = np.linalg.norm(out - ref) / np.linalg.norm(ref)
    print("rel err:", rel)
